# revision 1
# baseline (speedup 1.0000x reference)
"""Trainium2 Bass kernel for nn_Loss2_53996328845453 (segment_reduce).

Computes a multi-term image loss over B=16 samples of 512x512 images:
  total = 10*L_exp + 1*L_tv + 10*L_color + 50*L_sem

Strategy (pure data parallel, B sharded 2-per-core across 8 cores):
  - Semantic/color terms: per-sample Gram matrix on the TensorEngine.
    X side (stationary, chunk-major fp16): [R0,R1,R2, I0,I1,I2, R0²,R1²,R2², 1]
    Y side (moving, map-major fp16):       [M0..M7, M0²..M7², 1]
    Q=8 chunks packed per matmul; weights are chunk-contiguous (80 cols),
    rhs streams map-major in natural order (inner dim stride-1). Only the
    diagonal chunk blocks of each PSUM accumulation are meaningful; the
    whole [80,136] block is dumped and the diagonal extracted on host.
  - All HBM loads via HWDGE fp32; fp16 casting fused into on-chip
    reshuffle (ACT) / copy (DVE) / square (ACT) passes.
  - Exposure: per-row-band 16-wide partial sums on VectorE; patch assembly
    and (Lp-E)² on host.
  - TV: band-batched shifted row loads + VectorE subtract + abs-reduce.
  - Final scalar assembly on host in float64 from tiny per-core outputs.
"""
import os
import sys

import numpy as np

try:
    import concourse.bacc as bacc  # noqa: F401
except ImportError:
    sys.path.insert(0, "/opt/trn_rl_repo")

from contextlib import ExitStack

import concourse.bacc as bacc
import concourse.tile as tile
from concourse import mybir
from concourse import bass_utils

# problem constants (hardcoded per spec)
B, NCORES = 16, 8
BLOC = B // NCORES            # 2 samples per core
H = W = 512
HW = H * W                    # 262144 px
K, C = 8, 3
P = 128                       # SBUF partitions / matmul contraction
FALL = HW // P                # 2048 chunks of 128 px per sample
NSLAB = 4
F = FALL // NSLAB             # 512 chunks per slab
XC, YC = 10, 17               # gram columns per chunk (X stationary, Y moving)
Q = 8                         # chunks packed per matmul
NMM = F // Q                  # matmuls per slab
E_EXP = 0.6
PATCH = 16
L_EXP_W, L_TV_W, L_COLOR_W, L_SEM_W = 10.0, 1.0, 10.0, 50.0

f32 = mybir.dt.float32
f16 = mybir.dt.float16

_NC_CACHE = {}
LAST_RESULTS = None


def _build_nc():
    nc = bacc.Bacc("TRN2")
    L_d = nc.dram_tensor("L_loc", [BLOC, 1, H, W], f32, kind="ExternalInput")
    R_d = nc.dram_tensor("R_loc", [BLOC, C, H, W], f32, kind="ExternalInput")
    I_d = nc.dram_tensor("I_loc", [BLOC, C, H, W], f32, kind="ExternalInput")
    M_d = nc.dram_tensor("M_loc", [BLOC, K, H, W], f32, kind="ExternalInput")
    # constant bidiagonal shift matrix for vertical TV diffs on the PE
    S_d = nc.dram_tensor("shift_d", [P, P], f32, kind="ExternalInput")
    # full [80,136] gram block per sample (host extracts chunk-diagonal)
    gram_o = nc.dram_tensor(
        "gram_o", [BLOC, Q * XC, Q * YC], f32, kind="ExternalOutput"
    )
    # combined L-path output: [:, 0:128] exposure partials,
    # [:, 128:132] vertical TV band sums, [:, 132:136] horizontal TV,
    # [:, 136] band-boundary vertical sums (rows 0:3), rest pad
    lout_o = nc.dram_tensor("lout_o", [BLOC, P, 144], f32, kind="ExternalOutput")

    with ExitStack() as ctx:
        tc = ctx.enter_context(tile.TileContext(nc))
        xsp = ctx.enter_context(tc.tile_pool(name="xsp", bufs=3))
        msp = ctx.enter_context(tc.tile_pool(name="msp", bufs=3))
        xcp = ctx.enter_context(tc.tile_pool(name="xcp", bufs=3))
        yp = ctx.enter_context(tc.tile_pool(name="yp", bufs=3))
        lp = ctx.enter_context(tc.tile_pool(name="lp", bufs=1))
        sp = ctx.enter_context(tc.tile_pool(name="sp", bufs=2))
        op = ctx.enter_context(tc.tile_pool(name="op", bufs=2))
        cs = ctx.enter_context(tc.tile_pool(name="cs", bufs=1))
        pp = ctx.enter_context(tc.tile_pool(name="pp", bufs=2, space="PSUM"))
        vp = ctx.enter_context(tc.tile_pool(name="vp", bufs=2, space="PSUM"))

        Ssb = cs.tile([P, P], f32)
        nc.gpsimd.dma_start(out=Ssb, in_=S_d[:])

        for b in range(BLOC):
            # flat per-map HBM views: [128, nmaps, 2048]
            Rf = R_d[b].rearrange("c h w -> c (h w)").rearrange(
                "c (p f) -> p c f", p=P
            )
            If = I_d[b].rearrange("c h w -> c (h w)").rearrange(
                "c (p f) -> p c f", p=P
            )
            Mf = M_d[b].rearrange("k h w -> k (h w)").rearrange(
                "k (p f) -> p k f", p=P
            )

            psum_g = pp.tile([P, Q * YC], f32, tag="psum_g")

            # ---- L path: exposure partials + TV partials (band-batched)
            Lb = L_d[b, 0]  # [512, 512]
            Lbands = Lb.rearrange("(r p) w -> p r w", p=P)          # [128,4,512]
            ot = op.tile([P, 144], f32, tag="ot")
            Lt = lp.tile([P, 4, W], f32, tag="Lt")
            nc.gpsimd.dma_start(out=Lt, in_=Lbands)
            # band-boundary rows for vertical diffs (127,128),(255,256),(383,384)
            Ba = lp.tile([P, W], f32, tag="Ba")
            Bb = lp.tile([P, W], f32, tag="Bb")
            bnd = Lb.rearrange("(r p) w -> r p w", p=P)  # [4,128,512]
            nc.gpsimd.dma_start(out=Ba[0:3, :], in_=bnd[0:3, 127, :])
            nc.gpsimd.dma_start(out=Bb[0:3, :], in_=bnd[1:4, 0, :])
            # exposure: 16-wide sums along W -> [128, 4, 32] into ot[:,0:128]
            nc.vector.reduce_sum(
                ot[:, 0:128].rearrange("p (r c) -> p r c", r=4),
                Lt.rearrange("p r (g x) -> p r g x", x=PATCH),
                axis=mybir.AxisListType.X,
            )
            # horizontal TV: one wide sub + one wide abs-reduce
            dh = sp.tile([P, 4, W], f16, tag="dh")
            nc.vector.tensor_sub(
                dh[:, :, 0 : W - 1], Lt[:, :, 1:W], Lt[:, :, 0 : W - 1]
            )
            nc.vector.tensor_reduce(
                ot[:, 132:136],
                dh[:, :, 0 : W - 1],
                axis=mybir.AxisListType.X,
                op=mybir.AluOpType.add,
                apply_absolute_value=True,
            )
            # vertical TV within bands: PE bidiagonal shift (exact fp32),
            # row 127 of each product is zero (S col 127 is zero).
            for r in range(4):
                psum_v = vp.tile([P, W], f32, tag="psum_v")
                nc.tensor.matmul(
                    psum_v, lhsT=Ssb, rhs=Lt[:, r, :], start=True, stop=True
                )
                nc.vector.tensor_reduce(
                    ot[:, 128 + r : 129 + r],
                    psum_v,
                    axis=mybir.AxisListType.X,
                    op=mybir.AluOpType.add,
                    apply_absolute_value=True,
                )
            # vertical TV across band boundaries (3 rows)
            nc.vector.memset(ot[:, 136:144], 0.0)
            dv = sp.tile([P, W], f32, tag="dv")
            nc.vector.tensor_sub(dv[0:3, :], Bb[0:3, :], Ba[0:3, :])
            nc.vector.tensor_reduce(
                ot[0:3, 136:137],
                dv[0:3, :],
                axis=mybir.AxisListType.X,
                op=mybir.AluOpType.add,
                apply_absolute_value=True,
            )
            nc.sync.dma_start(out=lout_o[b], in_=ot)

            # tapered slabs: shorter final slabs shrink the tail
            # dependency chain after the last input bytes arrive
            bounds = [0, 512, 1024, 1536, 1792, 2048]
            for s in range(len(bounds) - 1):
                sl = slice(bounds[s], bounds[s + 1])
                Fs = bounds[s + 1] - bounds[s]

                # ---- X side: fp32 staging -> chunk-major fp16 stationary
                Xs = xsp.tile([P, 6, Fs], f32, tag="Xs")
                nc.gpsimd.dma_start(out=Xs[:, 0:3, :], in_=Rf[:, :, sl])
                nc.gpsimd.dma_start(out=Xs[:, 3:6, :], in_=If[:, :, sl])
                Xc = xcp.tile([P, Fs, XC], f16, tag="Xc")
                # reshuffle + cast on ACT
                nc.scalar.copy(Xc[:, :, 0:6], Xs.rearrange("p c f -> p f c"))
                # R^2 lanes (6:9) from R lanes (0:3), fp16 on DVE
                nc.vector.tensor_mul(Xc[:, :, 6:9], Xc[:, :, 0:3], Xc[:, :, 0:3])
                nc.vector.memset(Xc[:, :, 9], 1.0)

                # ---- Y side: fp32 staging -> map-major fp16 moving
                Ms = msp.tile([P, K, Fs], f32, tag="Ms")
                nc.sync.dma_start(out=Ms, in_=Mf[:, :, sl])
                Y = yp.tile([P, YC, Fs], f16, tag="Y")
                nc.vector.tensor_copy(Y[:, 0:8, :], Ms)  # cast copy on DVE
                nc.scalar.activation(                    # square + cast on ACT
                    Y[:, 8:16, :], Ms,
                    mybir.ActivationFunctionType.Square,
                )
                nc.vector.memset(Y[:, 16, :], 1.0)

                # ---- packed matmuls: Q chunks per instruction
                for m in range(Fs // Q):
                    g = bounds[s] // Q + m
                    j0 = m * Q
                    nc.tensor.matmul(
                        psum_g[0 : Q * XC, :],
                        lhsT=Xc[:, j0 : j0 + Q, :],
                        rhs=Y[:, :, j0 : j0 + Q],
                        start=(g == 0),
                        stop=(g == FALL // Q - 1),
                    )

            # ---- evacuate gram: aligned PSUM copies, one DMA per sample
            gram_sb = op.tile([P, Q * YC], f32, tag="gram_sb")
            nc.scalar.copy(gram_sb[0:32, :], psum_g[0:32, :])
            nc.scalar.copy(gram_sb[32:64, :], psum_g[32:64, :])
            nc.scalar.copy(gram_sb[64 : Q * XC, :], psum_g[64 : Q * XC, :])
            nc.sync.dma_start(out=gram_o[b], in_=gram_sb[0 : Q * XC, :])

    nc.finalize()
    return nc


def _get_nc():
    if "nc" not in _NC_CACHE:
        _NC_CACHE["nc"] = _build_nc()
    return _NC_CACHE["nc"]


def kernel(L, R, I_enh, semantic_masks):
    global LAST_RESULTS
    nc = _get_nc()

    # bidiagonal shift matrix: out[m] = L[m+1] - L[m] for m < 127
    S = np.zeros((P, P), dtype=np.float32)
    for m in range(P - 1):
        S[m + 1, m] = 1.0
        S[m, m] = -1.0

    in_maps = []
    for i in range(NCORES):
        sl = slice(BLOC * i, BLOC * (i + 1))
        in_maps.append(
            {
                "L_loc": np.ascontiguousarray(L[sl], dtype=np.float32),
                "R_loc": np.ascontiguousarray(R[sl], dtype=np.float32),
                "I_loc": np.ascontiguousarray(I_enh[sl], dtype=np.float32),
                "M_loc": np.ascontiguousarray(
                    semantic_masks[sl], dtype=np.float32
                ),
                "shift_d": S,
            }
        )

    res = bass_utils.run_bass_kernel_spmd(
        nc, in_maps, core_ids=list(range(NCORES))
    )
    LAST_RESULTS = res

    # ---- host-side combine in float64
    exp_acc = 0.0
    tv_acc_v = 0.0
    tv_acc_h = 0.0
    col_acc = 0.0
    sem_acc = 0.0
    for core in range(NCORES):
        o = res.results[core]
        gram_d = o["gram_o"].astype(np.float64)  # [BLOC, 80, 136]
        lout = o["lout_o"].astype(np.float64)    # [BLOC, P, 144]
        for b in range(BLOC):
            # diagonal extraction: value[q, xc, yc] = dump[10q+xc, 8yc+q]
            blocks = np.einsum(
                "qxyq->xy", gram_d[b].reshape(Q, XC, YC, Q)
            )  # summed over q: [XC, YC]
            g = blocks
            # X rows: 0:3 R, 3:6 I, 6:9 R^2, 9 ones
            # Y cols: 0:8 M, 8:16 M^2, 16 ones
            sRM = g[0:3, 0:8]        # [c, k]
            sRM2 = g[0:3, 8:16]
            sR2M2 = g[6:9, 8:16]
            sumI = g[3:6, 16]
            nvec = g[9, 0:8] + 1e-6
            sM2 = g[9, 8:16]
            mean = sRM / nvec[None, :]
            var = (sR2M2 - 2.0 * mean * sRM2 + mean * mean * sM2[None, :]).sum(
                axis=0
            ) / nvec
            sem_acc += var.sum()

            mI = sumI / HW
            col_acc += (
                (mI[0] - mI[1]) ** 2 + (mI[0] - mI[2]) ** 2 + (mI[1] - mI[2]) ** 2
            )

            # exposure: [p, (r, pw)] -> rows (h = 128*r + p) -> 16x16 patches
            expo = lout[b, :, 0:128].reshape(P, 4, 32)
            rows = expo.transpose(1, 0, 2).reshape(H, 32)
            patch = rows.reshape(32, PATCH, 32).sum(axis=1)
            Lp = patch / (PATCH * PATCH)
            exp_acc += ((Lp - E_EXP) ** 2).sum()

            tv_acc_v += lout[b, :, 128:132].sum() + lout[b, :, 136].sum()
            tv_acc_h += lout[b, :, 132:136].sum()

    L_exp = exp_acc / (B * 32 * 32)
    L_tv = tv_acc_v / (B * 1 * (H - 1) * W) + tv_acc_h / (B * 1 * H * (W - 1))
    L_color = col_acc / B
    L_sem = sem_acc / B
    total = (
        L_EXP_W * L_exp + L_TV_W * L_tv + L_COLOR_W * L_color + L_SEM_W * L_sem
    )
    return np.float32(total)

